# revision 21
# baseline (speedup 1.0000x reference)
"""Trainium2 Bass kernel for BasicMoE.

Reference computation (N=8192 tokens, D=1024 in, O=1024 out, E=8 experts):
    gates = softmax(x @ Wg + bg)                        # [N, E]
    out   = sum_e gates[:, e] * (x @ We[e] + be[e])     # [N, O]

Strategy: data-parallel over tokens. Each of the 8 NeuronCores gets a
1024-token shard of x plus the full (replicated) expert/gating weights and
computes its shard of the output. No collectives.

Per-core kernel (all matmuls bf16 inputs, f32 PSUM accumulate):
  - x shard is pre-transposed on host to xt[p, k*1024 + n] = x[n, k*128+p]
    so 128x128 lhsT tiles slice straight out of SBUF.
  - gating: z[t] = x_t @ Wg + bg via PE, softmax on ACT/DVE
    (exp with accum_out gives the row sums for free).
  - main: for e, t: psum[t,j] = sum_k xt_tile.T @ We_tile; then one fused
    DVE op acc = psum * g[:, e] + acc   (scalar_tensor_tensor).
  - bias: gT = transpose(g) on PE, psum_b = gT.T @ be (= g @ be), added
    into acc at the end.
"""

import numpy as np
import ml_dtypes

N_TOKENS = 8192
D = 1024   # in dim
O = 1024   # out dim
E = 8      # experts
NCORES = 8
NLOC = N_TOKENS // NCORES   # 1024 tokens per core
KT = D // 128               # 8 k-chunks
TT = NLOC // 128            # 8 token chunks
JT = O // 512               # 2 out chunks

BF16 = ml_dtypes.bfloat16

_CACHE = {}


def _build():
    """Build + compile the per-core Bass graph (same graph on all 8 cores)."""
    import concourse.bass as bass
    import concourse.mybir as mybir
    import concourse.tile as tile
    from concourse import bacc
    from concourse.masks import make_identity

    dt = mybir.dt
    f32 = dt.float32
    bf16 = dt.bfloat16
    Alu = mybir.AluOpType

    nc = bacc.Bacc(
        "TRN2",
        target_bir_lowering=False,
        debug=False,
        enable_asserts=False,
        num_devices=NCORES,
    )

    xt_d = nc.dram_tensor("xt", [128, KT * NLOC], bf16, kind="ExternalInput").ap()
    we_d = nc.dram_tensor("Wep", [E, 128, KT * O], bf16, kind="ExternalInput").ap()
    be_d = nc.dram_tensor("bep", [E, O], bf16, kind="ExternalInput").ap()
    wg_d = nc.dram_tensor("Wgp", [128, KT * E], bf16, kind="ExternalInput").ap()
    bg_d = nc.dram_tensor("bgp", [1, E], bf16, kind="ExternalInput").ap()
    out_d = nc.dram_tensor("out", [NLOC, O], f32, kind="ExternalOutput").ap()

    with tile.TileContext(nc) as tc:
        with (
            tc.tile_pool(name="const", bufs=1) as cpool,
            tc.tile_pool(name="xp", bufs=1) as xpool,
            tc.tile_pool(name="wp", bufs=3) as wpool,
            tc.tile_pool(name="ap", bufs=1) as apool,
            tc.tile_pool(name="gp", bufs=1) as gpool,
        ):
            ident = cpool.tile([128, 128], bf16)
            make_identity(nc, ident[:])
            ones = cpool.tile([1, 128], bf16)
            nc.gpsimd.memset(ones[:], 1.0)
            # Small gating/bias constants go on the SWDGE (gpsimd) queue so
            # they don't serialize behind xt on the sync HWDGE ring.
            wg_sb = cpool.tile([128, KT * E], bf16)
            nc.gpsimd.dma_start(wg_sb[:], wg_d)
            bg_sb = cpool.tile([1, E], bf16)
            nc.gpsimd.dma_start(bg_sb[:], bg_d)
            be_sb = cpool.tile([E, O], bf16)
            nc.gpsimd.dma_start(be_sb[:], be_d)

            # xt first, split across both HWDGE rings so the two halves
            # stream concurrently; everything downstream needs it.
            xt = xpool.tile([128, KT * NLOC], bf16)
            half = KT * NLOC // 2
            nc.sync.dma_start(xt[:, :half], xt_d[:, :half])
            nc.scalar.dma_start(xt[:, half:], xt_d[:, half:])

            acc = apool.tile([128, TT * O], f32)

            g_f32 = gpool.tile([128, TT * E], f32)
            g_bf = gpool.tile([128, TT * E], bf16)
            gT = gpool.tile([E, NLOC], bf16)
            negm = gpool.tile([128, TT], f32)
            ssum = gpool.tile([128, TT], f32)
            rec = gpool.tile([128, TT], f32)

            def xt_tile(k, t):
                c = k * NLOC + t * 128
                return xt[:, c : c + 128]

            # Expert weights on the same sync ring as xt: HWDGE drains FIFO,
            # so xt gets full HBM bandwidth first, then We[0], We[1], ... in
            # exactly the order compute consumes them. Each expert arrives as
            # two j-half DMAs so e=0 can start on the first half.
            we_tiles = []
            for e in range(E):
                we_sb = wpool.tile([128, KT * O], bf16, tag="we", name=f"we{e}")
                src = we_d[e].rearrange("p (k j c) -> j p k c", k=KT, j=JT, c=512)
                dst = we_sb.rearrange("p (k j c) -> j p k c", k=KT, j=JT, c=512)
                for jh in range(JT):
                    nc.sync.dma_start(dst[jh], src[jh])
                we_tiles.append(we_sb)

            # ---- Phase A: gating logits + softmax --------------------------
            with tc.tile_pool(name="psA", bufs=2, space="PSUM") as psA:
                for t in range(TT):
                    zg = psA.tile([128, E], f32, tag="zg")
                    for k in range(KT):
                        nc.tensor.matmul(
                            zg[:],
                            xt_tile(k, t),
                            wg_sb[:, k * E : (k + 1) * E],
                            start=(k == 0),
                            stop=False,
                        )
                    # + bg (rank-1: ones[1,128].T @ bg[1,E])
                    nc.tensor.matmul(zg[:], ones[:], bg_sb[:], start=False, stop=True)

                    nm = negm[:, t : t + 1]
                    nc.vector.tensor_reduce(
                        nm, zg[:], axis=mybir.AxisListType.X, op=Alu.max, negate=True
                    )
                    gs = g_f32[:, t * E : (t + 1) * E]
                    nc.scalar.activation(
                        gs,
                        zg[:],
                        mybir.ActivationFunctionType.Exp,
                        bias=nm,
                        scale=1.0,
                        accum_out=ssum[:, t : t + 1],
                    )
                    nc.vector.reciprocal(rec[:, t : t + 1], ssum[:, t : t + 1])
                    nc.vector.tensor_scalar_mul(gs, gs, rec[:, t : t + 1])
                    nc.vector.tensor_copy(g_bf[:, t * E : (t + 1) * E], gs)

            # ---- Phase A2: bias term g @ be initializes acc ----------------
            # These matmuls + copies run inside the PE/DVE idle window while
            # We[0] is still streaming from HBM, keeping them off the main
            # matmul stream entirely.
            with (
                tc.tile_pool(name="psC", bufs=1, space="PSUM") as psC,
                tc.tile_pool(name="psD", bufs=2, space="PSUM") as psD,
            ):
                for t in range(TT):
                    trp = psC.tile([E, 128], bf16, tag="tr")
                    nc.tensor.transpose(
                        trp[:], g_bf[:, t * E : (t + 1) * E], ident[:]
                    )
                    nc.vector.tensor_copy(gT[:, t * 128 : (t + 1) * 128], trp[:])
                for t in range(TT):
                    for j in range(JT):
                        bp = psD.tile([128, 512], f32, tag="bp", name=f"bp{j}")
                        nc.tensor.matmul(
                            bp[:],
                            gT[:, t * 128 : (t + 1) * 128],
                            be_sb[:, j * 512 : (j + 1) * 512],
                            start=True,
                            stop=True,
                        )
                        nc.vector.tensor_copy(
                            acc[:, t * O + j * 512 : t * O + (j + 1) * 512], bp[:]
                        )

            # ---- Phase B: expert GEMMs + gated accumulate ------------------
            # acc already holds the bias term, so every expert (including
            # e=0) runs the same fused DVE accumulate acc = psum*g_e + acc.
            with tc.tile_pool(name="psB", bufs=6, space="PSUM") as psB:
                # e = 0: j-outer so the j=0 half of We[0] is consumed as soon
                # as its DMA lands, ~3us before the j=1 half.
                for j in range(JT):
                    we_sb = we_tiles[0]
                    for t in range(TT):
                        ps0 = psB.tile([128, 512], f32, tag="mm", name="mm0")
                        for k in range(KT):
                            nc.tensor.matmul(
                                ps0[:],
                                xt_tile(k, t),
                                we_sb[:, k * O + j * 512 : k * O + (j + 1) * 512],
                                start=(k == 0),
                                stop=(k == KT - 1),
                            )
                        a_sl = acc[:, t * O + j * 512 : t * O + (j + 1) * 512]
                        nc.vector.scalar_tensor_tensor(
                            a_sl, ps0[:], g_f32[:, t * E : t * E + 1], a_sl,
                            op0=Alu.mult, op1=Alu.add,
                        )

                for e in range(1, E):
                    we_sb = we_tiles[e]
                    last = e == E - 1
                    for t in range(TT):
                        ps = [
                            psB.tile([128, 512], f32, tag="mm", name=f"mm{j}")
                            for j in range(JT)
                        ]
                        for k in range(KT):
                            lhs = xt_tile(k, t)
                            for j in range(JT):
                                nc.tensor.matmul(
                                    ps[j][:],
                                    lhs,
                                    we_sb[:, k * O + j * 512 : k * O + (j + 1) * 512],
                                    start=(k == 0),
                                    stop=(k == KT - 1),
                                )
                        gcol = g_f32[:, t * E + e : t * E + e + 1]
                        for j in range(JT):
                            a_sl = acc[:, t * O + j * 512 : t * O + (j + 1) * 512]
                            nc.vector.scalar_tensor_tensor(
                                a_sl, ps[j][:], gcol, a_sl,
                                op0=Alu.mult, op1=Alu.add,
                            )
                            if last:
                                nc.sync.dma_start(
                                    out_d[
                                        t * 128 : (t + 1) * 128,
                                        j * 512 : (j + 1) * 512,
                                    ],
                                    a_sl,
                                )

    nc.compile()
    return nc


def _get_nc():
    if "nc" not in _CACHE:
        _CACHE["nc"] = _build()
    return _CACHE["nc"]


def _pack_inputs(x, We, be, Wg, bg):
    """Host-side packing: shard + pre-transpose + bf16 cast."""
    x = np.asarray(x, dtype=np.float32)
    We = np.asarray(We, dtype=np.float32)
    be = np.asarray(be, dtype=np.float32)
    Wg = np.asarray(Wg, dtype=np.float32)
    bg = np.asarray(bg, dtype=np.float32)

    # [p, k*O + o] = We[e][k*128+p, o]
    we_p = np.ascontiguousarray(
        We.reshape(E, KT, 128, O).transpose(0, 2, 1, 3).reshape(E, 128, KT * O)
    ).astype(BF16)
    be_p = be.astype(BF16)
    wg_p = np.ascontiguousarray(
        Wg.reshape(KT, 128, E).transpose(1, 0, 2).reshape(128, KT * E)
    ).astype(BF16)
    bg_p = bg.reshape(1, E).astype(BF16)

    in_maps = []
    for i in range(NCORES):
        xs = x[i * NLOC : (i + 1) * NLOC]          # [NLOC, D]
        # xt[p, k*NLOC + n] = xs[n, k*128+p]
        xt = np.ascontiguousarray(
            xs.T.reshape(KT, 128, NLOC).transpose(1, 0, 2).reshape(128, KT * NLOC)
        ).astype(BF16)
        in_maps.append(
            {"xt": xt, "Wep": we_p, "bep": be_p, "Wgp": wg_p, "bgp": bg_p}
        )
    return in_maps


def _run(inputs, trace=False):
    """Returns (y_full, BassKernelResults)."""
    from concourse.bass_utils import run_bass_kernel_spmd

    nc = _get_nc()
    in_maps = _pack_inputs(**inputs)
    res = run_bass_kernel_spmd(
        nc, in_maps, core_ids=list(range(NCORES)), trace=trace
    )
    y = np.concatenate(
        [res.results[i]["out"] for i in range(NCORES)], axis=0
    ).astype(np.float32)
    return y, res


def kernel(**inputs):
    y, _ = _run(inputs, trace=False)
    return y
